# revision 1
# baseline (speedup 1.0000x reference)
"""Confusion-matrix (joint histogram) kernel for Trainium2.

Math: out[b, i, j] = #{pixels p in batch b : yp[b,p] == i and y[b,p] == j}
for i, j in [0, 21). Inputs yp, y are [8, 2048, 2048] int32, values in [0, 21).

Per NeuronCore (core c processes batch c):
  - DMA int32 pixel chunks into SBUF, one ScalarE copy converts to bf16,
  - one-hot masks as {0,1} planes in matmul-ready interleaved layout
    (planes[p, blk*126 + i*6 + g]) via tensor_scalar(is_equal), classes
    split across VectorE and GpSimd,
  - joint counts via TensorE: confusion = onehot(yp)^T @ onehot(y), 6
    pixel-columns per matmul ([128, 126] x [128, 126]) accumulated into one
    PSUM [126, 126] f32 tile (exact integer counts < 2^24),
  - host extracts + sums the 6 diagonal 21x21 blocks.
"""

import numpy as np

C = 21                  # classes
G = 6                   # pixel-column groups per matmul (G*C = 126 <= 128)
M = G * C               # 126
P = 128                 # partitions
FP = 504                # plane-chunk columns per tensor (divisible by 6)
N_GP = 0                # mask classes on GpSimd (rest on DVE)
SENTINEL = 64           # int32 value outside [0, 21)
MASK_DT = "bf16"

_CACHE = {}


def _build(
    n_free,
    work_cols=None,
    repeat=1,
    skip_mm=False,
    n_cls=C,
    n_gp=N_GP,
    mask_dt=MASK_DT,
):
    import concourse.bacc as bacc
    import concourse.mybir as mybir
    import concourse.tile as tile
    from contextlib import nullcontext

    if work_cols is None:
        work_cols = n_free

    nc = bacc.Bacc(
        "TRN2",
        target_bir_lowering=False,
        debug=False,
        enable_asserts=False,
        num_devices=8,
    )
    yp = nc.dram_tensor("yp", [P, n_free], mybir.dt.int32, kind="ExternalInput").ap()
    y = nc.dram_tensor("y", [P, n_free], mybir.dt.int32, kind="ExternalInput").ap()
    out = nc.dram_tensor("out", [M, M], mybir.dt.float32, kind="ExternalOutput").ap()

    n_main = (work_cols // FP) * FP
    tail_cols = work_cols - n_main                   # < FP
    tail_pad = -tail_cols % G
    tail_w = tail_cols + tail_pad
    total_mms = (n_main // G) + (tail_w // G)

    mdt = {"bf16": mybir.dt.bfloat16, "fp8": mybir.dt.float8e4}[mask_dt]
    bf16 = mybir.dt.bfloat16
    f32 = mybir.dt.float32
    i32 = mybir.dt.int32
    Copy = mybir.ActivationFunctionType.Copy
    n_dve = max(0, n_cls - n_gp)

    with tile.TileContext(nc) as tc:
        with (
            tc.tile_pool(name="psum", bufs=1, space="PSUM") as psum_pool,
            tc.tile_pool(name="cat", bufs=3) as cat_pool,
            tc.tile_pool(name="planes", bufs=2) as plane_pool,
            tc.tile_pool(name="singles", bufs=1) as singles,
        ):
            acc = psum_pool.tile([M, M], f32)
            mm = 0
            rep_ctx = tc.For_i(0, repeat, 1) if repeat > 1 else nullcontext()

            with rep_ctx:

                def do_plane_chunk(cat32, w):
                    """cat32: [128, 2*w] int32 = [yp vals | y vals], w % 6 == 0.

                    planes[p, blk*126 + i*6 + g] = (vals[p, blk*6+g] == i),
                    blk in [0, 2*w/6). A-side = blks [0, w/6), B-side = rest.
                    Each matmul reads a contiguous [128, 126] slice.
                    """
                    nonlocal mm
                    nblk = 2 * w // G
                    cat16 = cat_pool.tile([P, 2 * FP], bf16, tag="cat16")
                    c16 = cat16[:, : 2 * w]
                    nc.scalar.activation(c16[:], cat32[:], Copy)
                    planes = plane_pool.tile([P, C * 2 * FP], mdt, tag="planes")
                    pl3 = planes[:, : nblk * M].rearrange("p (b f) -> p b f", f=M)
                    cat3 = c16[:].rearrange("p (b f) -> p b f", f=G)
                    for i in range(n_dve):
                        nc.vector.tensor_scalar(
                            pl3[:, :, i * G : (i + 1) * G],
                            cat3[:],
                            float(i),
                            None,
                            mybir.AluOpType.is_equal,
                        )
                    for i in range(n_dve, n_cls):
                        nc.gpsimd.tensor_scalar(
                            pl3[:, :, i * G : (i + 1) * G],
                            cat3[:],
                            float(i),
                            None,
                            mybir.AluOpType.is_equal,
                        )
                    half = (w // G) * M
                    for t in (range(0) if skip_mm else range(w // G)):
                        nc.tensor.matmul(
                            acc[:, :],
                            planes[:, t * M : (t + 1) * M],
                            planes[:, half + t * M : half + (t + 1) * M],
                            start=(mm == 0),
                            stop=(mm == total_mms - 1),
                        )
                        mm += 1

                off = 0
                while off < n_main:
                    cat32 = cat_pool.tile([P, 2 * FP], i32, tag="cat32")
                    nc.sync.dma_start(cat32[:, :FP], yp[:, off : off + FP])
                    nc.sync.dma_start(cat32[:, FP:], y[:, off : off + FP])
                    do_plane_chunk(cat32, FP)
                    off += FP

                if tail_cols:
                    ct = cat_pool.tile([P, 2 * FP], i32, tag="cat32")
                    ctw = ct[:, : 2 * tail_w]
                    if tail_pad:
                        nc.vector.memset(ctw[:], SENTINEL)
                    nc.sync.dma_start(
                        ctw[:, :tail_cols], yp[:, n_main : n_main + tail_cols]
                    )
                    nc.sync.dma_start(
                        ctw[:, tail_w : tail_w + tail_cols],
                        y[:, n_main : n_main + tail_cols],
                    )
                    do_plane_chunk(ctw, tail_w)

            assert skip_mm or mm == total_mms
            res = singles.tile([M, M], f32)
            if skip_mm:
                nc.vector.memset(res[:], 0.0)
            else:
                nc.vector.tensor_copy(res[:], acc[:, :])
            nc.sync.dma_start(out, res[:])

    nc.compile()
    return nc


def _get(n_free):
    if n_free not in _CACHE:
        _CACHE[n_free] = _build(n_free)
    return _CACHE[n_free]


def kernel(yp, y, res, n_classes, _trace=False):
    from concourse import bass_utils

    yp = np.ascontiguousarray(np.asarray(yp))
    y = np.ascontiguousarray(np.asarray(y))
    B = yp.shape[0]
    n_free = yp[0].size // P
    nc = _get(n_free)
    in_maps = [
        {"yp": yp[b].reshape(P, n_free), "y": y[b].reshape(P, n_free)}
        for b in range(B)
    ]
    r = bass_utils.run_bass_kernel_spmd(
        nc, in_maps, core_ids=list(range(B)), trace=_trace
    )
    outs = []
    for b in range(B):
        Pm = r.results[b]["out"].astype(np.float64)
        Rb = np.zeros((C, C), np.float64)
        for g in range(G):
            Rb += Pm[g::G, g::G]
        outs.append(Rb)
    res_np = np.stack(outs).astype(np.float32)
    if _trace:
        kernel._last_results = r
    return res_np

